# revision 21
# baseline (speedup 1.0000x reference)
"""Trainium2 Bass kernel for nn_KalmanGraphicalModel (gnn_message_passing).

The reference runs ITERS=100 iterations of a LINEAR 3-point stencil in time:
    x <- A' x_t + B' x_{t-1} + C' x_{t+1} + Gam y_t     (edge-replicated)
Because the update is linear and gamma is small, the composed 100-step
operator is a banded convolution with numerically tiny bandwidth D (~14 for
gamma=0.01):
    x_100[t] = sum_{|d|<=D} G_d x0[t+d] + V_d y[t+d]
So the whole problem collapses to ONE banded-matmul pass on device:
  - time axis folded 16-way into the partition dim (16 blocks x 8 rows = 128)
  - the stencil taps become 128x128 block-banded weight matrices; taps that
    cross a fold boundary land in neighbor-column streams (sigma = -S..S)
  - per 512-col tile: nsig x-matmuls + ceil(nsig/2) y-matmuls accumulate in
    PSUM (y sigmas are packed two-per-matmul: the y input is replicated on
    host into a 128-partition tensor whose bottom half is column-shifted)
All I/O is bf16 (the rel-err budget is 2e-2; bf16 end-to-end lands ~2e-3).
T is sharded across 8 cores; the first/last 128 columns (edge-rule
influenced + window zero-padding) are computed host-side on tiny strips.
"""
import os
import numpy as np

N, M, T, ITERS = 8, 4, 500000, 100
NCORES = 8
L = T // NCORES          # 62500 timesteps per core
FOLD = 16                # time-fold factor -> 16 blocks x 8 rows = 128 partitions
NC = 3908                # out cols per core: 16*3908 = 62528 >= 62500
EDGE = 128               # host-computed override width at the two true edges
STRIP = 384              # width of host edge strips
TAU = 1e-10              # tap truncation threshold (relative)

_PROGRAM_CACHE = {}


def _compose_taps(F, H, Q, R, gamma):
    """Banded composition of the 100 linear steps, in float64."""
    Qinv = np.linalg.inv(Q)
    Rinv = np.linalg.inv(R)
    negQinv = -Qinv
    FtQinv = F.T @ Qinv
    HtRinv = H.T @ Rinv
    Z1 = np.eye(N); Z1[0, 0] = 0.0
    Z2 = np.eye(N); Z2[-1, -1] = 0.0
    Ap = np.eye(N) + gamma * (negQinv @ Z1 - FtQinv @ Z2 @ F - HtRinv @ H)
    Bp = -gamma * (negQinv @ Z1 @ F)
    Cp = gamma * (FtQinv @ Z2)
    Gam = gamma * HtRinv

    K = ITERS
    G = np.zeros((2 * K + 1, N, N))
    V = np.zeros((2 * K + 1, N, M))
    G[K] = np.eye(N)
    for _ in range(K):
        Gn = np.einsum("ij,djk->dik", Ap, G)
        Gn[:-1] += np.einsum("ij,djk->dik", Bp, G[1:])
        Gn[1:] += np.einsum("ij,djk->dik", Cp, G[:-1])
        Vn = np.einsum("ij,djk->dik", Ap, V)
        Vn[:-1] += np.einsum("ij,djk->dik", Bp, V[1:])
        Vn[1:] += np.einsum("ij,djk->dik", Cp, V[:-1])
        Vn[K] += Gam
        G, V = Gn, Vn

    gmax = np.abs(G).max(axis=(1, 2))
    vmax = np.abs(V).max(axis=(1, 2))
    scale = max(gmax.max(), vmax.max())
    keep = np.where((gmax > TAU * scale) | (vmax > TAU * scale))[0]
    D = int(max(1, np.abs(keep - K).max()))
    return G, V, D, (Ap.astype(np.float32), Bp.astype(np.float32),
                     Cp.astype(np.float32), Gam.astype(np.float32))


def _blocks(CW, S):
    """Column blocks matching per-pair consumption: pair p consumes window
    columns [1024p, 1024p + 1024 + 2S)."""
    blocks = [(0, 1024 + 2 * S)]
    c = 1024 + 2 * S
    while c < CW:
        cn = min(1024, CW - c)
        if CW - (c + cn) < 128:
            cn = CW - c
        blocks.append((c, cn))
        c += cn
    return blocks


def _build_program(S):
    """Build + schedule the Bass/Tile program (cached per S)."""
    import concourse.bass as bass
    import concourse.tile as tile
    from concourse import bacc, mybir

    if S in _PROGRAM_CACHE:
        return _PROGRAM_CACHE[S]

    CW = NC + 2 * S
    nsig = 2 * S + 1
    g = (nsig + 1) // 2          # y sigma-pack: 2 sigmas per matmul
    f32 = mybir.dt.float32
    bf16 = mybir.dt.bfloat16
    WCOLS = (nsig + g + 1) * 128

    nc = bacc.Bacc("TRN2", target_bir_lowering=False, debug=False,
                   enable_asserts=False, num_devices=NCORES)
    xf = nc.dram_tensor("xf", [128, CW], bf16, kind="ExternalInput").ap()
    y2 = nc.dram_tensor("y2", [128, CW], bf16, kind="ExternalInput").ap()
    wall = nc.dram_tensor("wall", [128, WCOLS], bf16, kind="ExternalInput").ap()
    out = nc.dram_tensor("out", [128, NC], bf16, kind="ExternalOutput").ap()

    # tiles: pairs of 512-col psum tiles; last tile is the 324-col remainder
    TS = 512
    tiles = []
    c = 0
    while c < NC:
        tiles.append((c, min(TS, NC - c)))
        c += TS
    assert len(tiles) % 2 == 0
    pairs = [(tiles[2 * p], tiles[2 * p + 1]) for p in range(len(tiles) // 2)]

    with tile.TileContext(nc) as tc:
        with tc.tile_pool(name="consts", bufs=1) as consts, \
             tc.tile_pool(name="warmps", bufs=1, space="PSUM") as warmps, \
             tc.tile_pool(name="ps", bufs=4, space="PSUM") as ps_pool, \
             tc.tile_pool(name="outp", bufs=2) as outp:
            # --- PE warmup: dummy matmuls on a memset tile bridge the ~3us
            # between kernel start and the first input DMA's completion
            # semaphore, so the PE clock is fully ramped (and the systolic
            # pipeline hot) when real operands arrive ---
            dum = consts.tile([128, 256], bf16)
            nc.vector.memset(dum[:], 0)
            wps = warmps.tile([128, 256], f32)
            for _ in range(13):
                nc.tensor.matmul(wps[:], dum[:, 0:128], dum[:], start=True,
                                 stop=True)

            # --- loads: weights on scalar; x/y interleaved per consumption
            # order on the sync queue so SDMA bandwidth is spent in exactly
            # the order the PE consumes (no y-ahead-of-x contention).
            # The very first pieces (x-sigma weight block, half of x block 0)
            # are split out as small transfers so their completion semaphores
            # fire as early as possible. ---
            wsb = consts.tile([128, WCOLS], bf16)
            nc.scalar.dma_start(wsb[:, 0:128], wall[:, 0:128])
            nc.scalar.dma_start(wsb[:, 128:WCOLS], wall[:, 128:WCOLS])
            xsb = consts.tile([128, CW], bf16)
            ysb = consts.tile([128, CW], bf16)
            first = True
            for (c0, cn) in _blocks(CW, S):
                if first:
                    h = 512 + 2 * S + 2
                    nc.sync.dma_start(xsb[:, 0:h], xf[:, 0:h])
                    nc.sync.dma_start(xsb[:, h:cn], xf[:, h:cn])
                    first = False
                else:
                    nc.sync.dma_start(xsb[:, c0:c0 + cn], xf[:, c0:c0 + cn])
                nc.sync.dma_start(ysb[:, c0:c0 + cn], y2[:, c0:c0 + cn])

            # --- compute: pair-wise, weight-stationary inner order ---
            for (a0, an), (b0, bn) in pairs:
                psa = ps_pool.tile([128, TS], f32, tag="ps")
                psb = ps_pool.tile([128, TS], f32, tag="ps")
                for si in range(nsig):
                    nc.tensor.matmul(psa[:, :an],
                                     wsb[:, si * 128:(si + 1) * 128],
                                     xsb[:, a0 + si:a0 + si + an],
                                     start=(si == 0), stop=False)
                    nc.tensor.matmul(psb[:, :bn],
                                     wsb[:, si * 128:(si + 1) * 128],
                                     xsb[:, b0 + si:b0 + si + bn],
                                     start=(si == 0), stop=False)
                for j in range(g - 1):
                    w0 = (nsig + j) * 128
                    nc.tensor.matmul(psa[:, :an], wsb[:, w0:w0 + 128],
                                     ysb[:, a0 + j:a0 + j + an],
                                     start=False, stop=False)
                    nc.tensor.matmul(psb[:, :bn], wsb[:, w0:w0 + 128],
                                     ysb[:, b0 + j:b0 + j + bn],
                                     start=False, stop=False)
                # last y sigma has only 64 contract rows: run tile a on PE
                # array rows 0-63 concurrently with tile b on rows 64-127
                # (tile b reads the g-shifted bottom half of y, so its
                # column window shifts back by g)
                wA = (nsig + g - 1) * 128
                wB = (nsig + g) * 128
                nc.tensor.matmul(psb[:, :bn], wsb[64:128, wB:wB + 128],
                                 ysb[64:128, b0 - 1:b0 - 1 + bn],
                                 start=False, stop=True)
                nc.tensor.matmul(psa[:, :an], wsb[0:64, wA:wA + 128],
                                 ysb[0:64, a0 + g - 1:a0 + g - 1 + an],
                                 start=False, stop=True)
                # evacuate (fp32 psum -> bf16 sbuf) and store
                if bn == TS:
                    ob = outp.tile([128, 2 * TS], bf16, tag="ob")
                    nc.vector.tensor_copy(ob[:, :an], psa[:, :an])
                    nc.vector.tensor_copy(ob[:, TS:TS + bn], psb[:, :bn])
                    nc_w = an + bn
                    nc.scalar.dma_start(out[:, a0:a0 + nc_w], ob[:, :nc_w])
                else:
                    # split the final pair into two stores on different
                    # queues so their dispatches don't serialize; evacuate
                    # the very last tile (b) first so its store chain —
                    # the kernel's critical tail — starts earliest
                    obb = outp.tile([128, TS], bf16, tag="obb")
                    nc.vector.tensor_copy(obb[:, :bn], psb[:, :bn])
                    nc.sync.dma_start(out[:, b0:b0 + bn], obb[:, :bn])
                    oba = outp.tile([128, TS], bf16, tag="oba")
                    nc.vector.tensor_copy(oba[:, :an], psa[:, :an])
                    nc.scalar.dma_start(out[:, a0:a0 + an], oba[:, :an])
    nc.compile()
    _PROGRAM_CACHE[S] = nc
    return nc


def _fold(a, rows, CW):
    # a: (rows, 16*CW) -> (rows*16 partitions, CW); partition b*rows+r holds
    # times t = c*16+b
    return np.ascontiguousarray(
        a.reshape(rows, CW, FOLD).transpose(2, 0, 1).reshape(FOLD * rows, CW))


def _run_edge_strip(x0, y, Ap, Bp, Cp, Gam):
    # reference-style edge replication on both strip ends; only the true-edge
    # side of the strip is consumed, the other side's garbage stays >100 cols
    # away from the EDGE-wide region we keep.
    x = x0.copy()
    for _ in range(ITERS):
        xp = np.concatenate([x[:, :1], x[:, :-1]], axis=1)
        xf_ = np.concatenate([x[:, 1:], x[:, -1:]], axis=1)
        x = (Ap @ x + Bp @ xp + Cp @ xf_ + Gam @ y).astype(np.float32)
    return x


def kernel(xs, ys, F, H, Q, R, gamma):
    import ml_dtypes
    from concourse.bass_utils import run_bass_kernel_spmd

    bf16 = np.dtype(ml_dtypes.bfloat16)
    xs = np.asarray(xs, dtype=np.float32)
    ysv = np.asarray(ys, dtype=np.float32)
    F64 = np.asarray(F, dtype=np.float64)
    H64 = np.asarray(H, dtype=np.float64)
    Q64 = np.asarray(Q, dtype=np.float64)
    R64 = np.asarray(R, dtype=np.float64)
    gv = float(np.asarray(gamma))

    G, V, D, mats32 = _compose_taps(F64, H64, Q64, R64, gv)
    S = (D + FOLD - 1) // FOLD
    assert S <= 7, f"bandwidth D={D} too large for single-pass kernel"
    CW = NC + 2 * S
    nsig = 2 * S + 1
    g = (nsig + 1) // 2

    # ---- weights ----
    K = ITERS
    WX = np.zeros((nsig, 128, 128), dtype=np.float32)
    WY = np.zeros((2 * g, 64, 128), dtype=np.float32)
    for si in range(nsig):
        sig = si - S
        for bo in range(FOLD):
            for bi in range(FOLD):
                d = sig * FOLD + bi - bo
                if abs(d) > D:
                    continue
                WX[si, bi * 8:bi * 8 + 8, bo * 8:bo * 8 + 8] = G[K + d].T
                WY[si, bi * 4:bi * 4 + 4, bo * 8:bo * 8 + 8] = V[K + d].T
    # wall layout: nsig x-blocks of [128,128], then g y-blocks of [128,128]
    # where y-block j rows 0:64 = WY[j], rows 64:128 = WY[j+g]; final extra
    # block holds WY[g-1] in rows 64:128 for the row-tiled concurrent matmul
    wall = np.zeros((128, (nsig + g + 1) * 128), dtype=np.float32)
    for si in range(nsig):
        wall[:, si * 128:(si + 1) * 128] = WX[si]
    for j in range(g):
        w0 = (nsig + j) * 128
        wall[0:64, w0:w0 + 128] = WY[j]
        wall[64:128, w0:w0 + 128] = WY[j + g]
    wB = (nsig + g) * 128
    wall[64:128, wB:wB + 128] = WY[g - 1]

    # ---- per-core folded input windows ----
    pad = FOLD * S
    padR = pad + (FOLD * NC - L)          # right overhang of core 7's window
    xs_p = np.zeros((N, T + pad + padR), dtype=np.float32)
    ys_p = np.zeros((M, T + pad + padR), dtype=np.float32)
    xs_p[:, pad:pad + T] = xs
    ys_p[:, pad:pad + T] = ysv
    wall_b = wall.astype(bf16)
    in_maps = []
    for i in range(NCORES):
        o = i * L
        yfold = _fold(ys_p[:, o:o + FOLD * CW], M, CW)      # [64, CW]
        y2 = np.zeros((128, CW), dtype=np.float32)
        y2[0:64] = yfold
        y2[64:128, :CW - g] = yfold[:, g:]                   # shifted by g cols
        in_maps.append({
            "xf": _fold(xs_p[:, o:o + FOLD * CW], N, CW).astype(bf16),
            "y2": y2.astype(bf16),
            "wall": wall_b,
        })

    nc = _build_program(S)
    trace = bool(int(os.environ.get("KALMAN_TRACE", "0")))
    res = run_bass_kernel_spmd(nc, in_maps, core_ids=list(range(NCORES)),
                               trace=trace)
    if trace and res.exec_time_ns is not None:
        print(f"HW exec time: {res.exec_time_ns} ns")
        print(f"HW exec time mean: {res.mean_exec_time_ns} ns")

    out_full = np.empty((N, T), dtype=np.float32)
    for i in range(NCORES):
        o = i * L
        Out = np.asarray(res.results[i]["out"]).astype(np.float32)  # (128, NC)
        unf = Out.reshape(FOLD, N, NC).transpose(1, 2, 0).reshape(N, FOLD * NC)
        out_full[:, o:o + L] = unf[:, :L]

    # ---- host edge strips (exact edge-replication dynamics) ----
    Ap32, Bp32, Cp32, Gam32 = mats32
    left = _run_edge_strip(xs[:, :STRIP], ysv[:, :STRIP],
                           Ap32, Bp32, Cp32, Gam32)
    right = _run_edge_strip(xs[:, -STRIP:], ysv[:, -STRIP:],
                            Ap32, Bp32, Cp32, Gam32)
    out_full[:, :EDGE] = left[:, :EDGE]
    out_full[:, -EDGE:] = right[:, -EDGE:]
    return out_full


# revision 24
# speedup vs baseline: 1.0158x; 1.0158x over previous
"""Trainium2 Bass kernel for nn_KalmanGraphicalModel (gnn_message_passing).

The reference runs ITERS=100 iterations of a LINEAR 3-point stencil in time:
    x <- A' x_t + B' x_{t-1} + C' x_{t+1} + Gam y_t     (edge-replicated)
Because the update is linear and gamma is small, the composed 100-step
operator is a banded convolution with numerically tiny bandwidth D (~14 for
gamma=0.01):
    x_100[t] = sum_{|d|<=D} G_d x0[t+d] + V_d y[t+d]
So the whole problem collapses to ONE banded-matmul pass on device:
  - time axis folded 16-way into the partition dim (16 blocks x 8 rows = 128)
  - the stencil taps become 128x128 block-banded weight matrices; taps that
    cross a fold boundary land in neighbor-column streams (sigma = -S..S)
  - per 512-col tile: nsig x-matmuls + ceil(nsig/2) y-matmuls accumulate in
    PSUM (y sigmas are packed two-per-matmul: the y input is replicated on
    host into a 128-partition tensor whose bottom half is column-shifted)
All I/O is bf16 (the rel-err budget is 2e-2; bf16 end-to-end lands ~2e-3).
T is sharded across 8 cores; the first/last 128 columns (edge-rule
influenced + window zero-padding) are computed host-side on tiny strips.
"""
import os
import numpy as np

N, M, T, ITERS = 8, 4, 500000, 100
NCORES = 8
L = T // NCORES          # 62500 timesteps per core
FOLD = 16                # time-fold factor -> 16 blocks x 8 rows = 128 partitions
NC = 3908                # out cols per core: 16*3908 = 62528 >= 62500
EDGE = 128               # host-computed override width at the two true edges
STRIP = 384              # width of host edge strips
TAU = 1e-10              # tap truncation threshold (relative)

_PROGRAM_CACHE = {}


def _compose_taps(F, H, Q, R, gamma):
    """Banded composition of the 100 linear steps, in float64."""
    Qinv = np.linalg.inv(Q)
    Rinv = np.linalg.inv(R)
    negQinv = -Qinv
    FtQinv = F.T @ Qinv
    HtRinv = H.T @ Rinv
    Z1 = np.eye(N); Z1[0, 0] = 0.0
    Z2 = np.eye(N); Z2[-1, -1] = 0.0
    Ap = np.eye(N) + gamma * (negQinv @ Z1 - FtQinv @ Z2 @ F - HtRinv @ H)
    Bp = -gamma * (negQinv @ Z1 @ F)
    Cp = gamma * (FtQinv @ Z2)
    Gam = gamma * HtRinv

    K = ITERS
    G = np.zeros((2 * K + 1, N, N))
    V = np.zeros((2 * K + 1, N, M))
    G[K] = np.eye(N)
    for _ in range(K):
        Gn = np.einsum("ij,djk->dik", Ap, G)
        Gn[:-1] += np.einsum("ij,djk->dik", Bp, G[1:])
        Gn[1:] += np.einsum("ij,djk->dik", Cp, G[:-1])
        Vn = np.einsum("ij,djk->dik", Ap, V)
        Vn[:-1] += np.einsum("ij,djk->dik", Bp, V[1:])
        Vn[1:] += np.einsum("ij,djk->dik", Cp, V[:-1])
        Vn[K] += Gam
        G, V = Gn, Vn

    gmax = np.abs(G).max(axis=(1, 2))
    vmax = np.abs(V).max(axis=(1, 2))
    scale = max(gmax.max(), vmax.max())
    keep = np.where((gmax > TAU * scale) | (vmax > TAU * scale))[0]
    D = int(max(1, np.abs(keep - K).max()))
    return G, V, D, (Ap.astype(np.float32), Bp.astype(np.float32),
                     Cp.astype(np.float32), Gam.astype(np.float32))


def _blocks(CW, S):
    """Column blocks matching per-pair consumption: pair p consumes window
    columns [1024p, 1024p + 1024 + 2S)."""
    blocks = [(0, 1024 + 2 * S)]
    c = 1024 + 2 * S
    while c < CW:
        cn = min(1024, CW - c)
        if CW - (c + cn) < 128:
            cn = CW - c
        blocks.append((c, cn))
        c += cn
    return blocks


def _build_program(S):
    """Build + schedule the Bass/Tile program (cached per S)."""
    import concourse.bass as bass
    import concourse.tile as tile
    from concourse import bacc, mybir

    rowtile = not bool(int(os.environ.get("KALMAN_NO_ROWTILE", "0")))
    key = (S, rowtile)
    if key in _PROGRAM_CACHE:
        return _PROGRAM_CACHE[key]

    CW = NC + 2 * S
    nsig = 2 * S + 1
    g = (nsig + 1) // 2          # y sigma-pack: 2 sigmas per matmul
    f32 = mybir.dt.float32
    bf16 = mybir.dt.bfloat16
    WCOLS = (nsig + g + 1) * 128

    nc = bacc.Bacc("TRN2", target_bir_lowering=False, debug=False,
                   enable_asserts=False, num_devices=NCORES)
    xf = nc.dram_tensor("xf", [128, CW], bf16, kind="ExternalInput").ap()
    y2 = nc.dram_tensor("y2", [128, CW], bf16, kind="ExternalInput").ap()
    wall = nc.dram_tensor("wall", [128, WCOLS], bf16, kind="ExternalInput").ap()
    out = nc.dram_tensor("out", [128, NC], bf16, kind="ExternalOutput").ap()

    # tiles: pairs of 512-col psum tiles; last tile is the 324-col remainder
    TS = 512
    tiles = []
    c = 0
    while c < NC:
        tiles.append((c, min(TS, NC - c)))
        c += TS
    assert len(tiles) % 2 == 0
    pairs = [(tiles[2 * p], tiles[2 * p + 1]) for p in range(len(tiles) // 2)]

    with tile.TileContext(nc) as tc:
        with tc.tile_pool(name="consts", bufs=1) as consts, \
             tc.tile_pool(name="warmps", bufs=1, space="PSUM") as warmps, \
             tc.tile_pool(name="ps", bufs=4, space="PSUM") as ps_pool, \
             tc.tile_pool(name="outp", bufs=2) as outp:
            # --- PE warmup: dummy matmuls on a memset tile bridge the ~3us
            # between kernel start and the first input DMA's completion
            # semaphore, so the PE clock is fully ramped (and the systolic
            # pipeline hot) when real operands arrive ---
            dum = consts.tile([128, 256], bf16)
            nc.vector.memset(dum[:], 0)
            wps = warmps.tile([128, 256], f32)
            for _ in range(13):
                nc.tensor.matmul(wps[:], dum[:, 0:128], dum[:], start=True,
                                 stop=True)

            # --- loads: weights on scalar; x/y interleaved per consumption
            # order on the sync queue so SDMA bandwidth is spent in exactly
            # the order the PE consumes (no y-ahead-of-x contention).
            # The very first pieces (x-sigma weight block, half of x block 0)
            # are split out as small transfers so their completion semaphores
            # fire as early as possible. ---
            wsb = consts.tile([128, WCOLS], bf16)
            nc.scalar.dma_start(wsb[:, 0:128], wall[:, 0:128])
            nc.scalar.dma_start(wsb[:, 128:WCOLS], wall[:, 128:WCOLS])
            xsb = consts.tile([128, CW], bf16)
            ysb = consts.tile([128, CW], bf16)
            first = True
            for (c0, cn) in _blocks(CW, S):
                if first:
                    h = 512 + 2 * S + 2
                    nc.sync.dma_start(xsb[:, 0:h], xf[:, 0:h])
                    nc.sync.dma_start(xsb[:, h:cn], xf[:, h:cn])
                    first = False
                else:
                    nc.sync.dma_start(xsb[:, c0:c0 + cn], xf[:, c0:c0 + cn])
                nc.sync.dma_start(ysb[:, c0:c0 + cn], y2[:, c0:c0 + cn])

            # --- compute: pair-wise, weight-stationary inner order ---
            for (a0, an), (b0, bn) in pairs:
                psa = ps_pool.tile([128, TS], f32, tag="ps")
                psb = ps_pool.tile([128, TS], f32, tag="ps")
                for si in range(nsig):
                    nc.tensor.matmul(psa[:, :an],
                                     wsb[:, si * 128:(si + 1) * 128],
                                     xsb[:, a0 + si:a0 + si + an],
                                     start=(si == 0), stop=False)
                    nc.tensor.matmul(psb[:, :bn],
                                     wsb[:, si * 128:(si + 1) * 128],
                                     xsb[:, b0 + si:b0 + si + bn],
                                     start=(si == 0), stop=False)
                nj = g - 1 if rowtile else g
                for j in range(nj):
                    w0 = (nsig + j) * 128
                    nc.tensor.matmul(psa[:, :an], wsb[:, w0:w0 + 128],
                                     ysb[:, a0 + j:a0 + j + an],
                                     start=False, stop=(j == g - 1))
                    nc.tensor.matmul(psb[:, :bn], wsb[:, w0:w0 + 128],
                                     ysb[:, b0 + j:b0 + j + bn],
                                     start=False, stop=(j == g - 1))
                if rowtile:
                    # last y sigma has only 64 contract rows: run tile a on
                    # PE array rows 0-63 concurrently with tile b on rows
                    # 64-127 (tile b reads the g-shifted bottom half of y,
                    # so its column window shifts back by g)
                    wA = (nsig + g - 1) * 128
                    wB = (nsig + g) * 128
                    nc.tensor.matmul(psb[:, :bn], wsb[64:128, wB:wB + 128],
                                     ysb[64:128, b0 - 1:b0 - 1 + bn],
                                     start=False, stop=True)
                    nc.tensor.matmul(psa[:, :an], wsb[0:64, wA:wA + 128],
                                     ysb[0:64, a0 + g - 1:a0 + g - 1 + an],
                                     start=False, stop=True)
                # evacuate (fp32 psum -> bf16 sbuf) and store
                if bn == TS:
                    ob = outp.tile([128, 2 * TS], bf16, tag="ob")
                    nc.vector.tensor_copy(ob[:, :an], psa[:, :an])
                    nc.vector.tensor_copy(ob[:, TS:TS + bn], psb[:, :bn])
                    nc_w = an + bn
                    nc.scalar.dma_start(out[:, a0:a0 + nc_w], ob[:, :nc_w])
                else:
                    # split the final pair into two stores on different
                    # queues so their dispatches don't serialize; evacuate
                    # the very last tile (b) first so its store chain —
                    # the kernel's critical tail — starts earliest
                    obb = outp.tile([128, TS], bf16, tag="obb")
                    nc.vector.tensor_copy(obb[:, :bn], psb[:, :bn])
                    nc.sync.dma_start(out[:, b0:b0 + bn], obb[:, :bn])
                    oba = outp.tile([128, TS], bf16, tag="oba")
                    nc.vector.tensor_copy(oba[:, :an], psa[:, :an])
                    nc.scalar.dma_start(out[:, a0:a0 + an], oba[:, :an])
    nc.compile()
    _PROGRAM_CACHE[key] = nc
    return nc


def _fold(a, rows, CW):
    # a: (rows, 16*CW) -> (rows*16 partitions, CW); partition b*rows+r holds
    # times t = c*16+b
    return np.ascontiguousarray(
        a.reshape(rows, CW, FOLD).transpose(2, 0, 1).reshape(FOLD * rows, CW))


def _run_edge_strip(x0, y, Ap, Bp, Cp, Gam):
    # reference-style edge replication on both strip ends; only the true-edge
    # side of the strip is consumed, the other side's garbage stays >100 cols
    # away from the EDGE-wide region we keep.
    x = x0.copy()
    for _ in range(ITERS):
        xp = np.concatenate([x[:, :1], x[:, :-1]], axis=1)
        xf_ = np.concatenate([x[:, 1:], x[:, -1:]], axis=1)
        x = (Ap @ x + Bp @ xp + Cp @ xf_ + Gam @ y).astype(np.float32)
    return x


def kernel(xs, ys, F, H, Q, R, gamma):
    import ml_dtypes
    from concourse.bass_utils import run_bass_kernel_spmd

    bf16 = np.dtype(ml_dtypes.bfloat16)
    xs = np.asarray(xs, dtype=np.float32)
    ysv = np.asarray(ys, dtype=np.float32)
    F64 = np.asarray(F, dtype=np.float64)
    H64 = np.asarray(H, dtype=np.float64)
    Q64 = np.asarray(Q, dtype=np.float64)
    R64 = np.asarray(R, dtype=np.float64)
    gv = float(np.asarray(gamma))

    G, V, D, mats32 = _compose_taps(F64, H64, Q64, R64, gv)
    S = (D + FOLD - 1) // FOLD
    assert S <= 7, f"bandwidth D={D} too large for single-pass kernel"
    CW = NC + 2 * S
    nsig = 2 * S + 1
    g = (nsig + 1) // 2

    # ---- weights ----
    K = ITERS
    WX = np.zeros((nsig, 128, 128), dtype=np.float32)
    WY = np.zeros((2 * g, 64, 128), dtype=np.float32)
    for si in range(nsig):
        sig = si - S
        for bo in range(FOLD):
            for bi in range(FOLD):
                d = sig * FOLD + bi - bo
                if abs(d) > D:
                    continue
                WX[si, bi * 8:bi * 8 + 8, bo * 8:bo * 8 + 8] = G[K + d].T
                WY[si, bi * 4:bi * 4 + 4, bo * 8:bo * 8 + 8] = V[K + d].T
    # wall layout: nsig x-blocks of [128,128], then g y-blocks of [128,128]
    # where y-block j rows 0:64 = WY[j], rows 64:128 = WY[j+g]; final extra
    # block holds WY[g-1] in rows 64:128 for the row-tiled concurrent matmul
    wall = np.zeros((128, (nsig + g + 1) * 128), dtype=np.float32)
    for si in range(nsig):
        wall[:, si * 128:(si + 1) * 128] = WX[si]
    for j in range(g):
        w0 = (nsig + j) * 128
        wall[0:64, w0:w0 + 128] = WY[j]
        wall[64:128, w0:w0 + 128] = WY[j + g]
    wB = (nsig + g) * 128
    wall[64:128, wB:wB + 128] = WY[g - 1]

    # ---- per-core folded input windows ----
    pad = FOLD * S
    padR = pad + (FOLD * NC - L)          # right overhang of core 7's window
    xs_p = np.zeros((N, T + pad + padR), dtype=np.float32)
    ys_p = np.zeros((M, T + pad + padR), dtype=np.float32)
    xs_p[:, pad:pad + T] = xs
    ys_p[:, pad:pad + T] = ysv
    wall_b = wall.astype(bf16)
    in_maps = []
    for i in range(NCORES):
        o = i * L
        yfold = _fold(ys_p[:, o:o + FOLD * CW], M, CW)      # [64, CW]
        y2 = np.zeros((128, CW), dtype=np.float32)
        y2[0:64] = yfold
        y2[64:128, :CW - g] = yfold[:, g:]                   # shifted by g cols
        in_maps.append({
            "xf": _fold(xs_p[:, o:o + FOLD * CW], N, CW).astype(bf16),
            "y2": y2.astype(bf16),
            "wall": wall_b,
        })

    nc = _build_program(S)
    trace = bool(int(os.environ.get("KALMAN_TRACE", "0")))
    res = run_bass_kernel_spmd(nc, in_maps, core_ids=list(range(NCORES)),
                               trace=trace)
    if trace and res.exec_time_ns is not None:
        print(f"HW exec time: {res.exec_time_ns} ns")
        print(f"HW exec time mean: {res.mean_exec_time_ns} ns")

    out_full = np.empty((N, T), dtype=np.float32)
    for i in range(NCORES):
        o = i * L
        Out = np.asarray(res.results[i]["out"]).astype(np.float32)  # (128, NC)
        unf = Out.reshape(FOLD, N, NC).transpose(1, 2, 0).reshape(N, FOLD * NC)
        out_full[:, o:o + L] = unf[:, :L]

    # ---- host edge strips (exact edge-replication dynamics) ----
    Ap32, Bp32, Cp32, Gam32 = mats32
    left = _run_edge_strip(xs[:, :STRIP], ysv[:, :STRIP],
                           Ap32, Bp32, Cp32, Gam32)
    right = _run_edge_strip(xs[:, -STRIP:], ysv[:, -STRIP:],
                            Ap32, Bp32, Cp32, Gam32)
    out_full[:, :EDGE] = left[:, :EDGE]
    out_full[:, -EDGE:] = right[:, -EDGE:]
    return out_full


# revision 25
# speedup vs baseline: 1.0394x; 1.0233x over previous
"""Trainium2 Bass kernel for nn_KalmanGraphicalModel (gnn_message_passing).

The reference runs ITERS=100 iterations of a LINEAR 3-point stencil in time:
    x <- A' x_t + B' x_{t-1} + C' x_{t+1} + Gam y_t     (edge-replicated)
Because the update is linear and gamma is small, the composed 100-step
operator is a banded convolution with numerically tiny bandwidth D (~14 for
gamma=0.01):
    x_100[t] = sum_{|d|<=D} G_d x0[t+d] + V_d y[t+d]
So the whole problem collapses to ONE banded-matmul pass on device:
  - time axis folded 16-way into the partition dim (16 blocks x 8 rows = 128)
  - the stencil taps become 128x128 block-banded weight matrices; taps that
    cross a fold boundary land in neighbor-column streams (sigma = -S..S)
  - per 512-col tile: nsig x-matmuls + ceil(nsig/2) y-matmuls accumulate in
    PSUM (y sigmas are packed two-per-matmul: the y input is replicated on
    host into a 128-partition tensor whose bottom half is column-shifted)
All I/O is bf16 (the rel-err budget is 2e-2; bf16 end-to-end lands ~2e-3).
T is sharded across 8 cores; the first/last 128 columns (edge-rule
influenced + window zero-padding) are computed host-side on tiny strips.
"""
import os
import numpy as np

N, M, T, ITERS = 8, 4, 500000, 100
NCORES = 8
L = T // NCORES          # 62500 timesteps per core
FOLD = 16                # time-fold factor -> 16 blocks x 8 rows = 128 partitions
NC = 3908                # out cols per core: 16*3908 = 62528 >= 62500
EDGE = 128               # host-computed override width at the two true edges
STRIP = 384              # width of host edge strips
TAU = 1e-10              # tap truncation threshold (relative)

_PROGRAM_CACHE = {}


def _compose_taps(F, H, Q, R, gamma):
    """Banded composition of the 100 linear steps, in float64."""
    Qinv = np.linalg.inv(Q)
    Rinv = np.linalg.inv(R)
    negQinv = -Qinv
    FtQinv = F.T @ Qinv
    HtRinv = H.T @ Rinv
    Z1 = np.eye(N); Z1[0, 0] = 0.0
    Z2 = np.eye(N); Z2[-1, -1] = 0.0
    Ap = np.eye(N) + gamma * (negQinv @ Z1 - FtQinv @ Z2 @ F - HtRinv @ H)
    Bp = -gamma * (negQinv @ Z1 @ F)
    Cp = gamma * (FtQinv @ Z2)
    Gam = gamma * HtRinv

    K = ITERS
    G = np.zeros((2 * K + 1, N, N))
    V = np.zeros((2 * K + 1, N, M))
    G[K] = np.eye(N)
    for _ in range(K):
        Gn = np.einsum("ij,djk->dik", Ap, G)
        Gn[:-1] += np.einsum("ij,djk->dik", Bp, G[1:])
        Gn[1:] += np.einsum("ij,djk->dik", Cp, G[:-1])
        Vn = np.einsum("ij,djk->dik", Ap, V)
        Vn[:-1] += np.einsum("ij,djk->dik", Bp, V[1:])
        Vn[1:] += np.einsum("ij,djk->dik", Cp, V[:-1])
        Vn[K] += Gam
        G, V = Gn, Vn

    gmax = np.abs(G).max(axis=(1, 2))
    vmax = np.abs(V).max(axis=(1, 2))
    scale = max(gmax.max(), vmax.max())
    keep = np.where((gmax > TAU * scale) | (vmax > TAU * scale))[0]
    D = int(max(1, np.abs(keep - K).max()))
    return G, V, D, (Ap.astype(np.float32), Bp.astype(np.float32),
                     Cp.astype(np.float32), Gam.astype(np.float32))


def _blocks(CW, S):
    """Column blocks matching per-pair consumption: pair p consumes window
    columns [1024p, 1024p + 1024 + 2S)."""
    blocks = [(0, 1024 + 2 * S)]
    c = 1024 + 2 * S
    while c < CW:
        cn = min(1024, CW - c)
        if CW - (c + cn) < 128:
            cn = CW - c
        blocks.append((c, cn))
        c += cn
    return blocks


def _build_program(S):
    """Build + schedule the Bass/Tile program (cached per S)."""
    import concourse.bass as bass
    import concourse.tile as tile
    from concourse import bacc, mybir

    rowtile = not bool(int(os.environ.get("KALMAN_NO_ROWTILE", "0")))
    key = (S, rowtile)
    if key in _PROGRAM_CACHE:
        return _PROGRAM_CACHE[key]

    CW = NC + 2 * S
    nsig = 2 * S + 1
    g = (nsig + 1) // 2          # y sigma-pack: 2 sigmas per matmul
    f32 = mybir.dt.float32
    bf16 = mybir.dt.bfloat16
    WCOLS = (nsig + g + 1) * 128

    nc = bacc.Bacc("TRN2", target_bir_lowering=False, debug=False,
                   enable_asserts=False, num_devices=NCORES)
    xf = nc.dram_tensor("xf", [128, CW], bf16, kind="ExternalInput").ap()
    y2 = nc.dram_tensor("y2", [128, CW], bf16, kind="ExternalInput").ap()
    wall = nc.dram_tensor("wall", [128, WCOLS], bf16, kind="ExternalInput").ap()
    out = nc.dram_tensor("out", [128, NC], bf16, kind="ExternalOutput").ap()

    # tiles: pairs of 512-col psum tiles; last tile is the 324-col remainder
    TS = 512
    tiles = []
    c = 0
    while c < NC:
        tiles.append((c, min(TS, NC - c)))
        c += TS
    assert len(tiles) % 2 == 0
    pairs = [(tiles[2 * p], tiles[2 * p + 1]) for p in range(len(tiles) // 2)]

    with tile.TileContext(nc) as tc:
        with tc.tile_pool(name="consts", bufs=1) as consts, \
             tc.tile_pool(name="warmps", bufs=1, space="PSUM") as warmps, \
             tc.tile_pool(name="ps", bufs=4, space="PSUM") as ps_pool, \
             tc.tile_pool(name="outp", bufs=2) as outp:
            # --- PE warmup: dummy matmuls on a memset tile bridge the ~3us
            # between kernel start and the first input DMA's completion
            # semaphore, so the PE clock is fully ramped (and the systolic
            # pipeline hot) when real operands arrive ---
            dum = consts.tile([128, 256], bf16)
            nc.vector.memset(dum[:], 0)
            wps = warmps.tile([128, 256], f32)
            for _ in range(13):
                nc.tensor.matmul(wps[:], dum[:, 0:128], dum[:], start=True,
                                 stop=True)

            # --- loads: weights on scalar; x/y interleaved per consumption
            # order on the sync queue so SDMA bandwidth is spent in exactly
            # the order the PE consumes (no y-ahead-of-x contention).
            # NOTE: keep each logical block ONE dma_start — splitting the
            # first block was tried and made the first matmul LATER, because
            # wait-consolidation onto the first LDWEIGHTS makes it gate on
            # the later sub-transfer's completion semaphore too. ---
            wsb = consts.tile([128, WCOLS], bf16)
            nc.scalar.dma_start(wsb[:], wall[:])
            xsb = consts.tile([128, CW], bf16)
            ysb = consts.tile([128, CW], bf16)
            for (c0, cn) in _blocks(CW, S):
                nc.sync.dma_start(xsb[:, c0:c0 + cn], xf[:, c0:c0 + cn])
                nc.sync.dma_start(ysb[:, c0:c0 + cn], y2[:, c0:c0 + cn])

            # --- compute: pair-wise, weight-stationary inner order ---
            for (a0, an), (b0, bn) in pairs:
                psa = ps_pool.tile([128, TS], f32, tag="ps")
                psb = ps_pool.tile([128, TS], f32, tag="ps")
                for si in range(nsig):
                    nc.tensor.matmul(psa[:, :an],
                                     wsb[:, si * 128:(si + 1) * 128],
                                     xsb[:, a0 + si:a0 + si + an],
                                     start=(si == 0), stop=False)
                    nc.tensor.matmul(psb[:, :bn],
                                     wsb[:, si * 128:(si + 1) * 128],
                                     xsb[:, b0 + si:b0 + si + bn],
                                     start=(si == 0), stop=False)
                nj = g - 1 if rowtile else g
                for j in range(nj):
                    w0 = (nsig + j) * 128
                    nc.tensor.matmul(psa[:, :an], wsb[:, w0:w0 + 128],
                                     ysb[:, a0 + j:a0 + j + an],
                                     start=False, stop=(j == g - 1))
                    nc.tensor.matmul(psb[:, :bn], wsb[:, w0:w0 + 128],
                                     ysb[:, b0 + j:b0 + j + bn],
                                     start=False, stop=(j == g - 1))
                if rowtile:
                    # last y sigma has only 64 contract rows: run tile a on
                    # PE array rows 0-63 concurrently with tile b on rows
                    # 64-127 (tile b reads the g-shifted bottom half of y,
                    # so its column window shifts back by g)
                    wA = (nsig + g - 1) * 128
                    wB = (nsig + g) * 128
                    nc.tensor.matmul(psb[:, :bn], wsb[64:128, wB:wB + 128],
                                     ysb[64:128, b0 - 1:b0 - 1 + bn],
                                     start=False, stop=True)
                    nc.tensor.matmul(psa[:, :an], wsb[0:64, wA:wA + 128],
                                     ysb[0:64, a0 + g - 1:a0 + g - 1 + an],
                                     start=False, stop=True)
                # evacuate (fp32 psum -> bf16 sbuf) and store
                if bn == TS:
                    ob = outp.tile([128, 2 * TS], bf16, tag="ob")
                    nc.vector.tensor_copy(ob[:, :an], psa[:, :an])
                    nc.vector.tensor_copy(ob[:, TS:TS + bn], psb[:, :bn])
                    nc_w = an + bn
                    nc.scalar.dma_start(out[:, a0:a0 + nc_w], ob[:, :nc_w])
                else:
                    # split the final pair into two stores on different
                    # queues so their dispatches don't serialize; evacuate
                    # the very last tile (b) first so its store chain —
                    # the kernel's critical tail — starts earliest
                    obb = outp.tile([128, TS], bf16, tag="obb")
                    nc.vector.tensor_copy(obb[:, :bn], psb[:, :bn])
                    nc.sync.dma_start(out[:, b0:b0 + bn], obb[:, :bn])
                    oba = outp.tile([128, TS], bf16, tag="oba")
                    nc.vector.tensor_copy(oba[:, :an], psa[:, :an])
                    nc.scalar.dma_start(out[:, a0:a0 + an], oba[:, :an])
    nc.compile()
    _PROGRAM_CACHE[key] = nc
    return nc


def _fold(a, rows, CW):
    # a: (rows, 16*CW) -> (rows*16 partitions, CW); partition b*rows+r holds
    # times t = c*16+b
    return np.ascontiguousarray(
        a.reshape(rows, CW, FOLD).transpose(2, 0, 1).reshape(FOLD * rows, CW))


def _run_edge_strip(x0, y, Ap, Bp, Cp, Gam):
    # reference-style edge replication on both strip ends; only the true-edge
    # side of the strip is consumed, the other side's garbage stays >100 cols
    # away from the EDGE-wide region we keep.
    x = x0.copy()
    for _ in range(ITERS):
        xp = np.concatenate([x[:, :1], x[:, :-1]], axis=1)
        xf_ = np.concatenate([x[:, 1:], x[:, -1:]], axis=1)
        x = (Ap @ x + Bp @ xp + Cp @ xf_ + Gam @ y).astype(np.float32)
    return x


def kernel(xs, ys, F, H, Q, R, gamma):
    import ml_dtypes
    from concourse.bass_utils import run_bass_kernel_spmd

    bf16 = np.dtype(ml_dtypes.bfloat16)
    xs = np.asarray(xs, dtype=np.float32)
    ysv = np.asarray(ys, dtype=np.float32)
    F64 = np.asarray(F, dtype=np.float64)
    H64 = np.asarray(H, dtype=np.float64)
    Q64 = np.asarray(Q, dtype=np.float64)
    R64 = np.asarray(R, dtype=np.float64)
    gv = float(np.asarray(gamma))

    G, V, D, mats32 = _compose_taps(F64, H64, Q64, R64, gv)
    S = (D + FOLD - 1) // FOLD
    assert S <= 7, f"bandwidth D={D} too large for single-pass kernel"
    CW = NC + 2 * S
    nsig = 2 * S + 1
    g = (nsig + 1) // 2

    # ---- weights ----
    K = ITERS
    WX = np.zeros((nsig, 128, 128), dtype=np.float32)
    WY = np.zeros((2 * g, 64, 128), dtype=np.float32)
    for si in range(nsig):
        sig = si - S
        for bo in range(FOLD):
            for bi in range(FOLD):
                d = sig * FOLD + bi - bo
                if abs(d) > D:
                    continue
                WX[si, bi * 8:bi * 8 + 8, bo * 8:bo * 8 + 8] = G[K + d].T
                WY[si, bi * 4:bi * 4 + 4, bo * 8:bo * 8 + 8] = V[K + d].T
    # wall layout: nsig x-blocks of [128,128], then g y-blocks of [128,128]
    # where y-block j rows 0:64 = WY[j], rows 64:128 = WY[j+g]; final extra
    # block holds WY[g-1] in rows 64:128 for the row-tiled concurrent matmul
    wall = np.zeros((128, (nsig + g + 1) * 128), dtype=np.float32)
    for si in range(nsig):
        wall[:, si * 128:(si + 1) * 128] = WX[si]
    for j in range(g):
        w0 = (nsig + j) * 128
        wall[0:64, w0:w0 + 128] = WY[j]
        wall[64:128, w0:w0 + 128] = WY[j + g]
    wB = (nsig + g) * 128
    wall[64:128, wB:wB + 128] = WY[g - 1]

    # ---- per-core folded input windows ----
    pad = FOLD * S
    padR = pad + (FOLD * NC - L)          # right overhang of core 7's window
    xs_p = np.zeros((N, T + pad + padR), dtype=np.float32)
    ys_p = np.zeros((M, T + pad + padR), dtype=np.float32)
    xs_p[:, pad:pad + T] = xs
    ys_p[:, pad:pad + T] = ysv
    wall_b = wall.astype(bf16)
    in_maps = []
    for i in range(NCORES):
        o = i * L
        yfold = _fold(ys_p[:, o:o + FOLD * CW], M, CW)      # [64, CW]
        y2 = np.zeros((128, CW), dtype=np.float32)
        y2[0:64] = yfold
        y2[64:128, :CW - g] = yfold[:, g:]                   # shifted by g cols
        in_maps.append({
            "xf": _fold(xs_p[:, o:o + FOLD * CW], N, CW).astype(bf16),
            "y2": y2.astype(bf16),
            "wall": wall_b,
        })

    nc = _build_program(S)
    trace = bool(int(os.environ.get("KALMAN_TRACE", "0")))
    res = run_bass_kernel_spmd(nc, in_maps, core_ids=list(range(NCORES)),
                               trace=trace)
    if trace and res.exec_time_ns is not None:
        print(f"HW exec time: {res.exec_time_ns} ns")
        print(f"HW exec time mean: {res.mean_exec_time_ns} ns")

    out_full = np.empty((N, T), dtype=np.float32)
    for i in range(NCORES):
        o = i * L
        Out = np.asarray(res.results[i]["out"]).astype(np.float32)  # (128, NC)
        unf = Out.reshape(FOLD, N, NC).transpose(1, 2, 0).reshape(N, FOLD * NC)
        out_full[:, o:o + L] = unf[:, :L]

    # ---- host edge strips (exact edge-replication dynamics) ----
    Ap32, Bp32, Cp32, Gam32 = mats32
    left = _run_edge_strip(xs[:, :STRIP], ysv[:, :STRIP],
                           Ap32, Bp32, Cp32, Gam32)
    right = _run_edge_strip(xs[:, -STRIP:], ysv[:, -STRIP:],
                            Ap32, Bp32, Cp32, Gam32)
    out_full[:, :EDGE] = left[:, :EDGE]
    out_full[:, -EDGE:] = right[:, -EDGE:]
    return out_full
